# revision 7
# baseline (speedup 1.0000x reference)
"""Depthwise-masked 3x3 conv (eye-masked dense conv) on 8 TRN2 NeuronCores.

Problem: x (2,16,256,64,64) fp32, W (256,256,3,3) fp32; the reference masks W
with eye(C) so only W[c,c,:,:] survives -> depthwise 3x3 "same" conv.

Strategy (per core; data-parallel over the 32 (s,b) samples -> 4 samples/core):
  - channels on partitions: work tile = (sample, channel-block of 128) ->
    x tile [128, 64, 64]; 8 work tiles per core, routed per-tile between the
    PE path (fp32r diag-matmuls) and the DVE path (bf16 elementwise).
  - PE path: 9 taps as diagonal-stationary matmuls in fp32r accumulating in
    PSUM per 512-element bank chunk; ACT builds a W-padded copy and evicts.
  - DVE path (bf16, 2x mode): input loaded via SWDGE cast-DMA (f32->bf16);
    ACT builds X1 (W-shifted-by-one padded copy) so ALL column-shifted taps
    read 4B-aligned bf16 views and stay in the DVE 2x_1P perf mode; ACT does
    the center tap (activation-copy with per-partition scale); DVE runs the
    8 neighbor taps as scalar_tensor_tensor; SWDGE cast-DMA (bf16->f32) out.
  - rel tolerance is 2e-2; bf16 on half the tiles lands ~3e-3.
"""

import os
from contextlib import ExitStack

import numpy as np

import concourse.bass as bass
import concourse.tile as tile
from concourse import bacc, mybir
from concourse.bass_utils import run_bass_kernel_spmd

S, B, C, H, W_SP = 2, 16, 256, 64, 64
N_CORES = 8
N_SAMPLES = S * B                      # 32
SPC = N_SAMPLES // N_CORES             # 4 samples per core
NBLK = C // 128                        # 2 channel blocks
N_TILES = SPC * NBLK                   # 8 work tiles per core
WPAD = W_SP + 2                        # 66: zero col, 64 data cols, zero col
ROWS_PER_CHUNK = 8                     # 512 fp32 = one PSUM bank
HALF_CHUNKS = 4                        # chunks per half tile (4 banks)
HALF_ROWS = HALF_CHUNKS * ROWS_PER_CHUNK  # 32
HSPLIT = HALF_ROWS + 2                 # xc half-DMA split row

# center tap first: start=True matmul must cover the full bank
TAPS = [(0, 0), (-1, -1), (-1, 0), (-1, 1), (0, -1), (0, 1), (1, -1), (1, 0), (1, 1)]

_DVE_TILES_DEFAULT = "1,3,5,7"
DVE_TILES = frozenset(
    int(v) for v in os.environ.get("KERNEL_DVE_TILES", _DVE_TILES_DEFAULT).split(",")
    if v != ""
)

F32 = mybir.dt.float32
F32R = mybir.dt.float32r
BF16 = mybir.dt.bfloat16


def _emit_pe_half(nc, psum_pool, osb_pool, g, half, xp, out_d, wd_sb):
    """PE path for rows [32*half, 32*half+32) of work tile g."""
    psum = psum_pool.tile([128, HALF_CHUNKS, ROWS_PER_CHUNK, W_SP], F32, tag="psum")
    for t, (dh, dw) in enumerate(TAPS):
        lhsT = wd_sb[:, ((g % NBLK) * 9 + t) * 128:((g % NBLK) * 9 + t + 1) * 128]
        for q in range(HALF_CHUNKS):
            h0 = half * HALF_ROWS + q * ROWS_PER_CHUNK
            a = max(h0, -dh)                      # first valid output row
            b = min(h0 + ROWS_PER_CHUNK, H - dh)  # one past last valid row
            rhs = xp[:, a + dh:b + dh, 1 + dw:1 + dw + W_SP]
            out_ap = psum[:, q, a - h0:b - h0, :]
            nc.tensor.matmul(out_ap, lhsT, rhs,
                             start=(t == 0), stop=(t == len(TAPS) - 1))
    osb = osb_pool.tile([128, HALF_CHUNKS * 512], F32, tag="osb")
    nc.scalar.copy(osb[:], psum[:, :, :, :])
    nc.sync.dma_start(
        out_d[g * 128:(g + 1) * 128, half * HALF_ROWS:(half + 1) * HALF_ROWS, :],
        osb[:],
    )


def _emit_pe_tile(nc, pools, g, x_d, out_d, wd_sb, zf_sb):
    xc_pool, xp_pool, psum_pool, osb_pool = pools
    xc = xc_pool.tile([128, H, W_SP], F32R, tag="xc")
    nc.sync.dma_start(xc[:, 0:HSPLIT, :],
                      x_d[g * 128:(g + 1) * 128, 0:HSPLIT, :])
    nc.sync.dma_start(xc[:, HSPLIT:H, :],
                      x_d[g * 128:(g + 1) * 128, HSPLIT:H, :])
    # padded tile for the PE halves (plain f32r copies, no cast)
    xp = xp_pool.tile([128, H, WPAD], F32R, tag="xp")
    nc.scalar.copy(xp[:, :, 0:1], zf_sb[:])
    nc.scalar.copy(xp[:, :, WPAD - 1:WPAD], zf_sb[:])
    nc.scalar.copy(xp[:, 0:HSPLIT, 1:1 + W_SP], xc[:, 0:HSPLIT, :])
    nc.scalar.copy(xp[:, HSPLIT:H, 1:1 + W_SP], xc[:, HSPLIT:H, :])
    for half in range(2):
        _emit_pe_half(nc, psum_pool, osb_pool, g, half, xp, out_d, wd_sb)


def _emit_dve_tile(nc, pools, g, x_d, out_d, wv_sb, zb_sb):
    """bf16 DVE path for the whole work tile g (all 9 taps, 2x perf mode)."""
    x0_pool, x1_pool, odve_pool = pools
    cb = g % NBLK

    x0 = x0_pool.tile([128, H, W_SP], BF16, tag="x0")
    src = x_d[g * 128:(g + 1) * 128, :, :].bitcast(F32)
    nc.gpsimd.dma_start(x0[:], src)  # f32->bf16 cast during DMA

    # X1[h, 1+w] = x[h, w]; zero cols 0 and 65.  dw=-1 taps read X1[:, :, 0:64]
    # (byte offset 0), dw=+1 taps read X1[:, :, 2:66] (byte offset 4): both
    # 4B-aligned step-1 bf16 views -> DVE 2x_1P stays engaged.
    x1 = x1_pool.tile([128, H, WPAD], BF16, tag="x1")
    nc.scalar.copy(x1[:, :, 0:1], zb_sb[:])
    nc.scalar.copy(x1[:, :, WPAD - 1:WPAD], zb_sb[:])
    nc.scalar.copy(x1[:, 0:HSPLIT, 1:1 + W_SP], x0[:, 0:HSPLIT, :])
    nc.scalar.copy(x1[:, HSPLIT:H, 1:1 + W_SP], x0[:, HSPLIT:H, :])

    odve = odve_pool.tile([128, H, W_SP], BF16, tag="odve")
    # center tap on ACT: odve = w_c * x0 (activation copy with per-part scale)
    nc.scalar.mul(odve[:], x0[:], wv_sb[:, cb * 9:cb * 9 + 1])

    for t, (dh, dw) in enumerate(TAPS[1:], start=1):
        wv = wv_sb[:, cb * 9 + t:cb * 9 + t + 1]
        oa = max(0, -dh)          # first valid output row
        ob = H - max(0, dh)       # one past last valid output row
        if dw == 0:
            in_v = x0[:, oa + dh:ob + dh, :]
        elif dw == -1:
            in_v = x1[:, oa + dh:ob + dh, 0:W_SP]
        else:
            in_v = x1[:, oa + dh:ob + dh, 2:2 + W_SP]
        out_v = odve[:, oa:ob, :]
        nc.vector.scalar_tensor_tensor(
            out_v, in_v, wv, out_v,
            op0=mybir.AluOpType.mult, op1=mybir.AluOpType.add,
        )
    nc.gpsimd.dma_start(out_d[g * 128:(g + 1) * 128, :, :], odve[:])  # bf16->f32


def _build_program(dve_tiles):
    nc = bacc.Bacc("TRN2", target_bir_lowering=False, debug=False)
    # x and wd carry fp32 bytes but are declared float32r so the PE can
    # consume them directly (PE truncates the extra mantissa bits).
    x_d = nc.dram_tensor("x", [SPC * C, H, W_SP], F32R, kind="ExternalInput").ap()
    wd_d = nc.dram_tensor("wd", [128, NBLK * 9 * 128], F32R, kind="ExternalInput").ap()
    wv_d = nc.dram_tensor("wv", [128, NBLK * 9], F32, kind="ExternalInput").ap()
    out_d = nc.dram_tensor("out", [SPC * C, H, W_SP], F32, kind="ExternalOutput").ap()

    with tile.TileContext(nc) as tc:
        with ExitStack() as ctx:
            const_pool = ctx.enter_context(tc.tile_pool(name="const", bufs=1))
            wd_sb = const_pool.tile([128, NBLK * 9 * 128], F32R)
            nc.sync.dma_start(wd_sb[:], wd_d[:])
            wv_sb = const_pool.tile([128, NBLK * 9], F32)
            nc.sync.dma_start(wv_sb[:], wv_d[:])
            zf32 = const_pool.tile([128, H, 1], F32)
            nc.vector.memset(zf32[:], 0.0)
            zf_sb = const_pool.tile([128, H, 1], F32R)
            nc.vector.tensor_copy(zf_sb[:], zf32[:])  # fp32r zeros for pad cols
            zb_sb = const_pool.tile([128, H, 1], BF16)
            nc.vector.memset(zb_sb[:], 0.0)

            xc_pool = ctx.enter_context(tc.tile_pool(name="xc", bufs=3))
            xp_pool = ctx.enter_context(tc.tile_pool(name="xp", bufs=2))
            psum_pool = ctx.enter_context(tc.tile_pool(name="psum", bufs=2, space="PSUM"))
            osb_pool = ctx.enter_context(tc.tile_pool(name="osb", bufs=4))
            x0_pool = ctx.enter_context(tc.tile_pool(name="x0", bufs=3))
            x1_pool = ctx.enter_context(tc.tile_pool(name="x1", bufs=2))
            odve_pool = ctx.enter_context(tc.tile_pool(name="odve", bufs=3))
            pe_pools = (xc_pool, xp_pool, psum_pool, osb_pool)
            dve_pools = (x0_pool, x1_pool, odve_pool)

            for g in range(N_TILES):
                if g in dve_tiles:
                    _emit_dve_tile(nc, dve_pools, g, x_d, out_d, wv_sb, zb_sb)
                else:
                    _emit_pe_tile(nc, pe_pools, g, x_d, out_d, wd_sb, zf_sb)
    nc.compile()
    return nc


_prog_cache = {}


def _get_program():
    key = DVE_TILES
    if key not in _prog_cache:
        _prog_cache[key] = _build_program(key)
    return _prog_cache[key]


def _host_weights(W):
    wdiag = W[np.arange(C), np.arange(C)]          # [256, 3, 3]
    wd_host = np.zeros((128, NBLK * 9, 128), dtype=np.float32)
    wv_host = np.zeros((128, NBLK * 9), dtype=np.float32)
    r = np.arange(128)
    for cb in range(NBLK):
        for t, (dh, dw) in enumerate(TAPS):
            wd_host[r, cb * 9 + t, r] = wdiag[cb * 128 + r, dh + 1, dw + 1]
            wv_host[r, cb * 9 + t] = wdiag[cb * 128 + r, dh + 1, dw + 1]
    return wd_host.reshape(128, NBLK * 9 * 128), wv_host


def _in_maps(x, W):
    wd_host, wv_host = _host_weights(W)
    xs = x.reshape(N_SAMPLES, C, H, W_SP)
    return [
        {
            "x": np.ascontiguousarray(xs[i * SPC:(i + 1) * SPC]).reshape(SPC * C, H, W_SP),
            "wd": wd_host,
            "wv": wv_host,
        }
        for i in range(N_CORES)
    ]


def kernel(x: np.ndarray, W: np.ndarray) -> np.ndarray:
    x = np.ascontiguousarray(x, dtype=np.float32)
    W = np.ascontiguousarray(W, dtype=np.float32)
    assert x.shape == (S, B, C, H, W_SP)
    assert W.shape == (C, C, 3, 3)

    nc = _get_program()
    res = run_bass_kernel_spmd(nc, _in_maps(x, W), core_ids=list(range(N_CORES)))
    out = np.concatenate(
        [res.results[i]["out"].reshape(SPC, C, H, W_SP) for i in range(N_CORES)], axis=0
    )
    return out.reshape(S, B, C, H, W_SP)


# revision 9
# speedup vs baseline: 1.2246x; 1.2246x over previous
"""Depthwise-masked 3x3 conv (eye-masked dense conv) on 8 TRN2 NeuronCores.

Problem: x (2,16,256,64,64) fp32, W (256,256,3,3) fp32; the reference masks W
with eye(C) so only W[c,c,:,:] survives -> depthwise 3x3 "same" conv.

Strategy (per core; data-parallel over the 32 (s,b) samples -> 4 samples/core):
  - channels on partitions: work tile = (sample, channel-block of 128) ->
    x tile [128, 64, 64]; 8 work tiles per core, routed per-tile between the
    PE path (fp32r diag-matmuls) and the DVE path (bf16 elementwise).
  - PE path: 9 taps as diagonal-stationary matmuls in fp32r accumulating in
    PSUM per 512-element bank chunk; ACT builds a W-padded copy and evicts.
  - DVE path (bf16, 2x mode): input loaded via SWDGE cast-DMA (f32->bf16);
    ACT builds X1 (W-shifted-by-one padded copy) so ALL column-shifted taps
    read 4B-aligned bf16 views and stay in the DVE 2x_1P perf mode; ACT does
    the center tap (activation-copy with per-partition scale); DVE runs the
    8 neighbor taps as scalar_tensor_tensor; SWDGE cast-DMA (bf16->f32) out.
  - rel tolerance is 2e-2; bf16 on half the tiles lands ~3e-3.
"""

import os
from contextlib import ExitStack

import numpy as np

import concourse.bass as bass
import concourse.tile as tile
from concourse import bacc, mybir
from concourse.bass_utils import run_bass_kernel_spmd

S, B, C, H, W_SP = 2, 16, 256, 64, 64
N_CORES = 8
N_SAMPLES = S * B                      # 32
SPC = N_SAMPLES // N_CORES             # 4 samples per core
NBLK = C // 128                        # 2 channel blocks
N_TILES = SPC * NBLK                   # 8 work tiles per core
WPAD = W_SP + 2                        # 66: zero col, 64 data cols, zero col
ROWS_PER_CHUNK = 8                     # 512 fp32 = one PSUM bank
HALF_CHUNKS = 4                        # chunks per half tile (4 banks)
HALF_ROWS = HALF_CHUNKS * ROWS_PER_CHUNK  # 32
HSPLIT = HALF_ROWS + 2                 # xc half-DMA split row
HM1 = H                                # tmp tile rows (<= H)

# center tap first: start=True matmul must cover the full bank
TAPS = [(0, 0), (-1, -1), (-1, 0), (-1, 1), (0, -1), (0, 1), (1, -1), (1, 0), (1, 1)]

_DVE_TILES_DEFAULT = "1,4,6"
DVE_TILES = frozenset(
    int(v) for v in os.environ.get("KERNEL_DVE_TILES", _DVE_TILES_DEFAULT).split(",")
    if v != ""
)

F32 = mybir.dt.float32
F32R = mybir.dt.float32r
BF16 = mybir.dt.bfloat16


def _emit_pe_half(nc, psum_pool, osb_pool, g, half, xp, out_d, wd_sb):
    """PE path for rows [32*half, 32*half+32) of work tile g."""
    psum = psum_pool.tile([128, HALF_CHUNKS, ROWS_PER_CHUNK, W_SP], F32, tag="psum")
    for t, (dh, dw) in enumerate(TAPS):
        lhsT = wd_sb[:, ((g % NBLK) * 9 + t) * 128:((g % NBLK) * 9 + t + 1) * 128]
        for q in range(HALF_CHUNKS):
            h0 = half * HALF_ROWS + q * ROWS_PER_CHUNK
            a = max(h0, -dh)                      # first valid output row
            b = min(h0 + ROWS_PER_CHUNK, H - dh)  # one past last valid row
            rhs = xp[:, a + dh:b + dh, 1 + dw:1 + dw + W_SP]
            out_ap = psum[:, q, a - h0:b - h0, :]
            nc.tensor.matmul(out_ap, lhsT, rhs,
                             start=(t == 0), stop=(t == len(TAPS) - 1))
    osb = osb_pool.tile([128, HALF_CHUNKS * 512], F32, tag="osb")
    nc.scalar.copy(osb[:], psum[:, :, :, :])
    nc.sync.dma_start(
        out_d[g * 128:(g + 1) * 128, half * HALF_ROWS:(half + 1) * HALF_ROWS, :],
        osb[:],
    )


def _emit_pe_tile(nc, pools, g, x_d, out_d, wd_sb, zf_sb):
    xc_pool, xp_pool, psum_pool, osb_pool = pools
    xc = xc_pool.tile([128, H, W_SP], F32R, tag="xc")
    nc.sync.dma_start(xc[:, 0:HSPLIT, :],
                      x_d[g * 128:(g + 1) * 128, 0:HSPLIT, :])
    nc.sync.dma_start(xc[:, HSPLIT:H, :],
                      x_d[g * 128:(g + 1) * 128, HSPLIT:H, :])
    # padded tile for the PE halves (plain f32r copies, no cast)
    xp = xp_pool.tile([128, H, WPAD], F32R, tag="xp")
    nc.scalar.copy(xp[:, :, 0:1], zf_sb[:])
    nc.scalar.copy(xp[:, :, WPAD - 1:WPAD], zf_sb[:])
    nc.scalar.copy(xp[:, 0:HSPLIT, 1:1 + W_SP], xc[:, 0:HSPLIT, :])
    nc.scalar.copy(xp[:, HSPLIT:H, 1:1 + W_SP], xc[:, HSPLIT:H, :])
    for half in range(2):
        _emit_pe_half(nc, psum_pool, osb_pool, g, half, xp, out_d, wd_sb)


def _emit_dve_tile(nc, pools, g, x_d, out_d, wv_sb, zb_sb):
    """bf16 DVE path for the whole work tile g.

    scalar_tensor_tensor has no fast uop (always 1x), so each tap is
    tensor_scalar (4x mode) into a tmp followed by tensor_tensor add (2x).
    """
    x0_pool, x1_pool, tmp_pool, odve_pool = pools
    cb = g % NBLK

    x0 = x0_pool.tile([128, H, W_SP], BF16, tag="x0")
    src = x_d[g * 128:(g + 1) * 128, :, :].bitcast(F32)
    nc.gpsimd.dma_start(x0[:], src)  # f32->bf16 cast during DMA

    # X1[h, 1+w] = x[h, w]; zero cols 0 and 65.  dw=-1 taps read X1[:, :, 0:64]
    # (byte offset 0), dw=+1 taps read X1[:, :, 2:66] (byte offset 4): both
    # 4B-aligned step-1 bf16 views so the DVE 2x/4x perf modes stay engaged.
    x1 = x1_pool.tile([128, H, WPAD], BF16, tag="x1")
    nc.scalar.copy(x1[:, :, 0:1], zb_sb[:])
    nc.scalar.copy(x1[:, :, WPAD - 1:WPAD], zb_sb[:])
    nc.scalar.copy(x1[:, 0:HSPLIT, 1:1 + W_SP], x0[:, 0:HSPLIT, :])
    nc.scalar.copy(x1[:, HSPLIT:H, 1:1 + W_SP], x0[:, HSPLIT:H, :])

    odve = odve_pool.tile([128, H, W_SP], BF16, tag="odve")
    # center tap: odve = w_c * x0 (tensor_scalar, 4x)
    nc.vector.tensor_scalar(odve[:], x0[:], wv_sb[:, cb * 9:cb * 9 + 1], None,
                            mybir.AluOpType.mult)

    for t, (dh, dw) in enumerate(TAPS[1:], start=1):
        wv = wv_sb[:, cb * 9 + t:cb * 9 + t + 1]
        oa = max(0, -dh)          # first valid output row
        ob = H - max(0, dh)       # one past last valid output row
        if dw == 0:
            in_v = x0[:, oa + dh:ob + dh, :]
        elif dw == -1:
            in_v = x1[:, oa + dh:ob + dh, 0:W_SP]
        else:
            in_v = x1[:, oa + dh:ob + dh, 2:2 + W_SP]
        tmp = tmp_pool.tile([128, HM1, W_SP], BF16, tag="tmp")
        nr = ob - oa
        nc.vector.tensor_scalar(tmp[:, 0:nr, :], in_v, wv, None,
                                mybir.AluOpType.mult)
        out_v = odve[:, oa:ob, :]
        nc.vector.tensor_tensor(out_v, out_v, tmp[:, 0:nr, :],
                                op=mybir.AluOpType.add)
    nc.gpsimd.dma_start(out_d[g * 128:(g + 1) * 128, :, :], odve[:])  # bf16->f32


def _build_program(dve_tiles):
    nc = bacc.Bacc("TRN2", target_bir_lowering=False, debug=False)
    # x and wd carry fp32 bytes but are declared float32r so the PE can
    # consume them directly (PE truncates the extra mantissa bits).
    x_d = nc.dram_tensor("x", [SPC * C, H, W_SP], F32R, kind="ExternalInput").ap()
    wd_d = nc.dram_tensor("wd", [128, NBLK * 9 * 128], F32R, kind="ExternalInput").ap()
    wv_d = nc.dram_tensor("wv", [128, NBLK * 9], F32, kind="ExternalInput").ap()
    out_d = nc.dram_tensor("out", [SPC * C, H, W_SP], F32, kind="ExternalOutput").ap()

    with tile.TileContext(nc) as tc:
        with ExitStack() as ctx:
            const_pool = ctx.enter_context(tc.tile_pool(name="const", bufs=1))
            wd_sb = const_pool.tile([128, NBLK * 9 * 128], F32R)
            nc.sync.dma_start(wd_sb[:], wd_d[:])
            wv_sb = const_pool.tile([128, NBLK * 9], F32)
            nc.sync.dma_start(wv_sb[:], wv_d[:])
            zf32 = const_pool.tile([128, H, 1], F32)
            nc.vector.memset(zf32[:], 0.0)
            zf_sb = const_pool.tile([128, H, 1], F32R)
            nc.vector.tensor_copy(zf_sb[:], zf32[:])  # fp32r zeros for pad cols
            zb_sb = const_pool.tile([128, H, 1], BF16)
            nc.vector.memset(zb_sb[:], 0.0)

            xc_pool = ctx.enter_context(tc.tile_pool(name="xc", bufs=3))
            xp_pool = ctx.enter_context(tc.tile_pool(name="xp", bufs=2))
            psum_pool = ctx.enter_context(tc.tile_pool(name="psum", bufs=2, space="PSUM"))
            osb_pool = ctx.enter_context(tc.tile_pool(name="osb", bufs=3))
            x0_pool = ctx.enter_context(tc.tile_pool(name="x0", bufs=3))
            x1_pool = ctx.enter_context(tc.tile_pool(name="x1", bufs=2))
            tmp_pool = ctx.enter_context(tc.tile_pool(name="tmp", bufs=2))
            odve_pool = ctx.enter_context(tc.tile_pool(name="odve", bufs=2))
            pe_pools = (xc_pool, xp_pool, psum_pool, osb_pool)
            dve_pools = (x0_pool, x1_pool, tmp_pool, odve_pool)

            for g in range(N_TILES):
                if g in dve_tiles:
                    _emit_dve_tile(nc, dve_pools, g, x_d, out_d, wv_sb, zb_sb)
                else:
                    _emit_pe_tile(nc, pe_pools, g, x_d, out_d, wd_sb, zf_sb)
    nc.compile()
    return nc


_prog_cache = {}


def _get_program():
    key = DVE_TILES
    if key not in _prog_cache:
        _prog_cache[key] = _build_program(key)
    return _prog_cache[key]


def _host_weights(W):
    wdiag = W[np.arange(C), np.arange(C)]          # [256, 3, 3]
    wd_host = np.zeros((128, NBLK * 9, 128), dtype=np.float32)
    wv_host = np.zeros((128, NBLK * 9), dtype=np.float32)
    r = np.arange(128)
    for cb in range(NBLK):
        for t, (dh, dw) in enumerate(TAPS):
            wd_host[r, cb * 9 + t, r] = wdiag[cb * 128 + r, dh + 1, dw + 1]
            wv_host[r, cb * 9 + t] = wdiag[cb * 128 + r, dh + 1, dw + 1]
    return wd_host.reshape(128, NBLK * 9 * 128), wv_host


def _in_maps(x, W):
    wd_host, wv_host = _host_weights(W)
    xs = x.reshape(N_SAMPLES, C, H, W_SP)
    return [
        {
            "x": np.ascontiguousarray(xs[i * SPC:(i + 1) * SPC]).reshape(SPC * C, H, W_SP),
            "wd": wd_host,
            "wv": wv_host,
        }
        for i in range(N_CORES)
    ]


def kernel(x: np.ndarray, W: np.ndarray) -> np.ndarray:
    x = np.ascontiguousarray(x, dtype=np.float32)
    W = np.ascontiguousarray(W, dtype=np.float32)
    assert x.shape == (S, B, C, H, W_SP)
    assert W.shape == (C, C, 3, 3)

    nc = _get_program()
    res = run_bass_kernel_spmd(nc, _in_maps(x, W), core_ids=list(range(N_CORES)))
    out = np.concatenate(
        [res.results[i]["out"].reshape(SPC, C, H, W_SP) for i in range(N_CORES)], axis=0
    )
    return out.reshape(S, B, C, H, W_SP)


# revision 12
# speedup vs baseline: 1.4966x; 1.2221x over previous
"""Depthwise-masked 3x3 conv (eye-masked dense conv) on 8 TRN2 NeuronCores.

Problem: x (2,16,256,64,64) fp32, W (256,256,3,3) fp32; the reference masks W
with eye(C) so only W[c,c,:,:] survives -> depthwise 3x3 "same" conv.

Strategy (per core; data-parallel over the 32 (s,b) samples -> 4 samples/core):
  - channels on partitions: work tile = (sample, channel-block of 128) ->
    x tile [128, 64, 64]; 8 work tiles per core, routed per-tile between the
    PE path (fp32r diag-matmuls) and the DVE path (bf16 elementwise).
  - PE path: 9 taps as diagonal-stationary matmuls in fp32r accumulating in
    PSUM per 512-element bank chunk; ACT builds a W-padded copy and evicts.
  - DVE path (bf16, 2x mode): input loaded via SWDGE cast-DMA (f32->bf16);
    ACT builds X1 (W-shifted-by-one padded copy) so ALL column-shifted taps
    read 4B-aligned bf16 views and stay in the DVE 2x_1P perf mode; ACT does
    the center tap (activation-copy with per-partition scale); DVE runs the
    8 neighbor taps as scalar_tensor_tensor; SWDGE cast-DMA (bf16->f32) out.
  - rel tolerance is 2e-2; bf16 on half the tiles lands ~3e-3.
"""

import os
from contextlib import ExitStack

import numpy as np

import concourse.bass as bass
import concourse.tile as tile
from concourse import bacc, mybir
from concourse.bass_utils import run_bass_kernel_spmd

S, B, C, H, W_SP = 2, 16, 256, 64, 64
N_CORES = 8
N_SAMPLES = S * B                      # 32
SPC = N_SAMPLES // N_CORES             # 4 samples per core
NBLK = C // 128                        # 2 channel blocks
N_TILES = SPC * NBLK                   # 8 work tiles per core
WPAD = W_SP + 2                        # 66: zero col, 64 data cols, zero col
ROWS_PER_CHUNK = 8                     # 512 fp32 = one PSUM bank
HALF_CHUNKS = 4                        # chunks per half tile (4 banks)
HALF_ROWS = HALF_CHUNKS * ROWS_PER_CHUNK  # 32
HSPLIT = HALF_ROWS + 2                 # xc half-DMA split row
HM1 = H                                # tmp tile rows (<= H)

# center tap first: start=True matmul must cover the full bank
TAPS = [(0, 0), (-1, -1), (-1, 0), (-1, 1), (0, -1), (0, 1), (1, -1), (1, 0), (1, 1)]

_DVE_TILES_DEFAULT = "1,4,6"
DVE_TILES = frozenset(
    int(v) for v in os.environ.get("KERNEL_DVE_TILES", _DVE_TILES_DEFAULT).split(",")
    if v != ""
)

F32 = mybir.dt.float32
F32R = mybir.dt.float32r
BF16 = mybir.dt.bfloat16


def _emit_pe_half(nc, psum_pool, osb_pool, g, half, xp, out_d, wd_sb):
    """PE path for rows [32*half, 32*half+32) of work tile g."""
    psum = psum_pool.tile([128, HALF_CHUNKS, ROWS_PER_CHUNK, W_SP], F32, tag="psum")
    for t, (dh, dw) in enumerate(TAPS):
        lhsT = wd_sb[:, ((g % NBLK) * 9 + t) * 128:((g % NBLK) * 9 + t + 1) * 128]
        for q in range(HALF_CHUNKS):
            h0 = half * HALF_ROWS + q * ROWS_PER_CHUNK
            a = max(h0, -dh)                      # first valid output row
            b = min(h0 + ROWS_PER_CHUNK, H - dh)  # one past last valid row
            rhs = xp[:, a + dh:b + dh, 1 + dw:1 + dw + W_SP]
            out_ap = psum[:, q, a - h0:b - h0, :]
            nc.tensor.matmul(out_ap, lhsT, rhs,
                             start=(t == 0), stop=(t == len(TAPS) - 1))
    # evict to bf16 (halves ACT cost); SWDGE cast-store widens back to f32
    osb = osb_pool.tile([128, HALF_CHUNKS * 512], BF16, tag="osb")
    nc.scalar.copy(osb[:], psum[:, :, :, :])
    nc.gpsimd.dma_start(
        out_d[g * 128:(g + 1) * 128, half * HALF_ROWS:(half + 1) * HALF_ROWS, :],
        osb[:],
    )


def _emit_pe_tile(nc, pools, g, x_d, out_d, wd_sb, zf_sb):
    xc_pool, xp_pool, psum_pool, osb_pool = pools
    xc = xc_pool.tile([128, H, W_SP], F32R, tag="xc")
    nc.sync.dma_start(xc[:, 0:HSPLIT, :],
                      x_d[g * 128:(g + 1) * 128, 0:HSPLIT, :])
    nc.sync.dma_start(xc[:, HSPLIT:H, :],
                      x_d[g * 128:(g + 1) * 128, HSPLIT:H, :])
    # padded tile for the PE halves (plain f32r copies, no cast)
    xp = xp_pool.tile([128, H, WPAD], F32R, tag="xp")
    nc.scalar.copy(xp[:, :, 0:1], zf_sb[:])
    nc.scalar.copy(xp[:, :, WPAD - 1:WPAD], zf_sb[:])
    nc.scalar.copy(xp[:, 0:HSPLIT, 1:1 + W_SP], xc[:, 0:HSPLIT, :])
    nc.scalar.copy(xp[:, HSPLIT:H, 1:1 + W_SP], xc[:, HSPLIT:H, :])
    for half in range(2):
        _emit_pe_half(nc, psum_pool, osb_pool, g, half, xp, out_d, wd_sb)


def _emit_dve_tile(nc, pools, g, x_d, out_d, wv_sb, zb_sb):
    """bf16 DVE path for the whole work tile g.

    scalar_tensor_tensor has no fast uop (always 1x), so each tap is
    tensor_scalar (4x mode) into a tmp followed by tensor_tensor add (2x).
    """
    x0_pool, x1_pool, tmp_pool, tmp6_pool, odve_pool = pools
    cb = g % NBLK

    x0 = x0_pool.tile([128, H, W_SP], BF16, tag="x0")
    src = x_d[g * 128:(g + 1) * 128, :, :].bitcast(F32)
    nc.gpsimd.dma_start(x0[:], src)  # f32->bf16 cast during DMA

    # X1[h, 1+w] = x[h, w]; zero cols 0 and 65.  dw=-1 taps read X1[:, :, 0:64]
    # (byte offset 0), dw=+1 taps read X1[:, :, 2:66] (byte offset 4): both
    # 4B-aligned step-1 bf16 views so the DVE 2x/4x perf modes stay engaged.
    x1 = x1_pool.tile([128, H, WPAD], BF16, tag="x1")
    nc.scalar.copy(x1[:, :, 0:1], zb_sb[:])
    nc.scalar.copy(x1[:, :, WPAD - 1:WPAD], zb_sb[:])
    nc.scalar.copy(x1[:, 0:HSPLIT, 1:1 + W_SP], x0[:, 0:HSPLIT, :])
    nc.scalar.copy(x1[:, HSPLIT:H, 1:1 + W_SP], x0[:, HSPLIT:H, :])

    odve = odve_pool.tile([128, H, W_SP], BF16, tag="odve")
    # center tap: odve = w_c * x0 (tensor_scalar, 4x)
    nc.vector.tensor_scalar(odve[:], x0[:], wv_sb[:, cb * 9:cb * 9 + 1], None,
                            mybir.AluOpType.mult)

    # All tensor_scalar multiplies run DENSE from offset 0 (keeps the 4x_2P
    # perf mode, which needs full 8B phase); the dh/dw shifts move into the
    # tensor_tensor add's read view, which only needs 4B alignment for 2x.
    for t, (dh, dw) in enumerate(TAPS[1:], start=1):
        wv = wv_sb[:, cb * 9 + t:cb * 9 + t + 1]
        oa = max(0, -dh)          # first valid output row
        ob = H - max(0, dh)       # one past last valid output row
        if dw == 0:
            tmp = tmp_pool.tile([128, H, W_SP], BF16, tag="tmp")
            nc.vector.tensor_scalar(tmp[:], x0[:], wv, None,
                                    mybir.AluOpType.mult)
            in_v = tmp[:, oa + dh:ob + dh, :]
        else:
            tmp6 = tmp6_pool.tile([128, H, WPAD], BF16, tag="tmp6")
            nc.vector.tensor_scalar(tmp6[:], x1[:], wv, None,
                                    mybir.AluOpType.mult)
            col0 = 0 if dw == -1 else 2
            in_v = tmp6[:, oa + dh:ob + dh, col0:col0 + W_SP]
        out_v = odve[:, oa:ob, :]
        nc.vector.tensor_tensor(out_v, out_v, in_v, op=mybir.AluOpType.add)
    nc.gpsimd.dma_start(out_d[g * 128:(g + 1) * 128, :, :], odve[:])  # bf16->f32


def _build_program(dve_tiles):
    nc = bacc.Bacc("TRN2", target_bir_lowering=False, debug=False)
    # x and wd carry fp32 bytes but are declared float32r so the PE can
    # consume them directly (PE truncates the extra mantissa bits).
    x_d = nc.dram_tensor("x", [SPC * C, H, W_SP], F32R, kind="ExternalInput").ap()
    wd_d = nc.dram_tensor("wd", [128, NBLK * 9 * 128], F32R, kind="ExternalInput").ap()
    wv_d = nc.dram_tensor("wv", [128, NBLK * 9], F32, kind="ExternalInput").ap()
    out_d = nc.dram_tensor("out", [SPC * C, H, W_SP], F32, kind="ExternalOutput").ap()

    with tile.TileContext(nc) as tc:
        with ExitStack() as ctx:
            const_pool = ctx.enter_context(tc.tile_pool(name="const", bufs=1))
            wd_sb = const_pool.tile([128, NBLK * 9 * 128], F32R)
            nc.sync.dma_start(wd_sb[:], wd_d[:])
            wv_sb = const_pool.tile([128, NBLK * 9], F32)
            nc.sync.dma_start(wv_sb[:], wv_d[:])
            zf32 = const_pool.tile([128, H, 1], F32)
            nc.vector.memset(zf32[:], 0.0)
            zf_sb = const_pool.tile([128, H, 1], F32R)
            nc.vector.tensor_copy(zf_sb[:], zf32[:])  # fp32r zeros for pad cols
            zb_sb = const_pool.tile([128, H, 1], BF16)
            nc.vector.memset(zb_sb[:], 0.0)

            xc_pool = ctx.enter_context(tc.tile_pool(name="xc", bufs=3))
            xp_pool = ctx.enter_context(tc.tile_pool(name="xp", bufs=2))
            psum_pool = ctx.enter_context(tc.tile_pool(name="psum", bufs=2, space="PSUM"))
            osb_pool = ctx.enter_context(tc.tile_pool(name="osb", bufs=3))
            x0_pool = ctx.enter_context(tc.tile_pool(name="x0", bufs=3))
            x1_pool = ctx.enter_context(tc.tile_pool(name="x1", bufs=2))
            tmp_pool = ctx.enter_context(tc.tile_pool(name="tmp", bufs=2))
            tmp6_pool = ctx.enter_context(tc.tile_pool(name="tmp6", bufs=2))
            odve_pool = ctx.enter_context(tc.tile_pool(name="odve", bufs=2))
            pe_pools = (xc_pool, xp_pool, psum_pool, osb_pool)
            dve_pools = (x0_pool, x1_pool, tmp_pool, tmp6_pool, odve_pool)

            for g in range(N_TILES):
                if g in dve_tiles:
                    _emit_dve_tile(nc, dve_pools, g, x_d, out_d, wv_sb, zb_sb)
                else:
                    _emit_pe_tile(nc, pe_pools, g, x_d, out_d, wd_sb, zf_sb)
    nc.compile()
    return nc


_prog_cache = {}


def _get_program():
    key = DVE_TILES
    if key not in _prog_cache:
        _prog_cache[key] = _build_program(key)
    return _prog_cache[key]


def _host_weights(W):
    wdiag = W[np.arange(C), np.arange(C)]          # [256, 3, 3]
    wd_host = np.zeros((128, NBLK * 9, 128), dtype=np.float32)
    wv_host = np.zeros((128, NBLK * 9), dtype=np.float32)
    r = np.arange(128)
    for cb in range(NBLK):
        for t, (dh, dw) in enumerate(TAPS):
            wd_host[r, cb * 9 + t, r] = wdiag[cb * 128 + r, dh + 1, dw + 1]
            wv_host[r, cb * 9 + t] = wdiag[cb * 128 + r, dh + 1, dw + 1]
    return wd_host.reshape(128, NBLK * 9 * 128), wv_host


def _in_maps(x, W):
    wd_host, wv_host = _host_weights(W)
    xs = x.reshape(N_SAMPLES, C, H, W_SP)
    return [
        {
            "x": np.ascontiguousarray(xs[i * SPC:(i + 1) * SPC]).reshape(SPC * C, H, W_SP),
            "wd": wd_host,
            "wv": wv_host,
        }
        for i in range(N_CORES)
    ]


def kernel(x: np.ndarray, W: np.ndarray) -> np.ndarray:
    x = np.ascontiguousarray(x, dtype=np.float32)
    W = np.ascontiguousarray(W, dtype=np.float32)
    assert x.shape == (S, B, C, H, W_SP)
    assert W.shape == (C, C, 3, 3)

    nc = _get_program()
    res = run_bass_kernel_spmd(nc, _in_maps(x, W), core_ids=list(range(N_CORES)))
    out = np.concatenate(
        [res.results[i]["out"].reshape(SPC, C, H, W_SP) for i in range(N_CORES)], axis=0
    )
    return out.reshape(S, B, C, H, W_SP)


# revision 17
# speedup vs baseline: 1.5544x; 1.0386x over previous
"""Depthwise-masked 3x3 conv (eye-masked dense conv) on 8 TRN2 NeuronCores.

Problem: x (2,16,256,64,64) fp32, W (256,256,3,3) fp32; the reference masks W
with eye(C) so only W[c,c,:,:] survives -> depthwise 3x3 "same" conv.

Strategy (per core; data-parallel over the 32 (s,b) samples -> 4 samples/core):
  - channels on partitions: work tile = (sample, channel-block of 128) ->
    bf16 x tile [128, 64, 64] via SWDGE cast-DMA; 8 work tiles per core.
  - PE path: taps as diagonal-stationary bf16 matmuls accumulating in fp32
    PSUM per 512-element bank chunk; boundaries via clipped access patterns
    (bf16 ifmaps allow the odd-width views fp32r rejects).
  - DVE path: per tap, tensor_scalar multiply (4x perf mode, dense from
    offset 0) into a tmp, then tensor_tensor add (2x) with the dh/dw shift
    absorbed into the add's read view; X1 (W-shifted padded copy, built by
    ACT) keeps the column-shifted reads 4B-aligned.
  - hybrid tiles split taps: PE takes the 6 column-shifted taps, DVE takes
    the 3 dw=0 taps and adds its partial onto the evicted PSUM result.
  - rel tolerance is 2e-2; bf16 everywhere lands ~3.5e-3.
"""

import os
from contextlib import ExitStack

import numpy as np
import ml_dtypes

import concourse.bass as bass
import concourse.tile as tile
from concourse import bacc, mybir
from concourse.bass_utils import run_bass_kernel_spmd

S, B, C, H, W_SP = 2, 16, 256, 64, 64
N_CORES = 8
N_SAMPLES = S * B                      # 32
SPC = N_SAMPLES // N_CORES             # 4 samples per core
NBLK = C // 128                        # 2 channel blocks
N_TILES = SPC * NBLK                   # 8 work tiles per core
WPAD = W_SP + 2                        # 66: zero col, 64 data cols, zero col
ROWS_PER_CHUNK = 8                     # 512 fp32 = one PSUM bank
HALF_CHUNKS = 4                        # chunks per half tile (4 banks)
HALF_ROWS = HALF_CHUNKS * ROWS_PER_CHUNK  # 32
HSPLIT = HALF_ROWS + 2                 # x0 half-DMA split row

# center tap first: the start=True matmul covers the full bank
TAPS = [(0, 0), (-1, -1), (-1, 0), (-1, 1), (0, -1), (0, 1), (1, -1), (1, 0), (1, 1)]
DW0_TAPS = [(0, 0), (-1, 0), (1, 0)]                       # DVE side of hybrids
DWX_TAPS = [(-1, -1), (-1, 1), (0, -1), (0, 1), (1, -1), (1, 1)]  # PE side

_DVE_TILES_DEFAULT = "1,6"
_HYB_TILES_DEFAULT = "3,4"
DVE_TILES = frozenset(
    int(v) for v in os.environ.get("KERNEL_DVE_TILES", _DVE_TILES_DEFAULT).split(",")
    if v != ""
)
HYB_TILES = frozenset(
    int(v) for v in os.environ.get("KERNEL_HYB_TILES", _HYB_TILES_DEFAULT).split(",")
    if v != ""
)

F32 = mybir.dt.float32
BF16 = mybir.dt.bfloat16


def _tap_slot(g, t):
    return (g % NBLK) * 9 + t


def _emit_pe_taps(nc, psum, g, half, x0, wd_sb, taps):
    """Diag-matmul the given taps for rows [32*half, ...) into psum.

    Clipped access patterns handle all boundaries: start=True clears the
    whole bank's has_written bits, each element's first writer overwrites,
    later ones accumulate — order independent, so partial-width taps are
    fine as long as every element is covered by some tap.
    """
    for i, (dh, dw) in enumerate(taps):
        t = TAPS.index((dh, dw))
        lhsT = wd_sb[:, _tap_slot(g, t) * 128:(_tap_slot(g, t) + 1) * 128]
        co0 = max(0, -dw)              # first valid output col
        co1 = W_SP - max(0, dw)        # one past last valid output col
        for q in range(HALF_CHUNKS):
            h0 = half * HALF_ROWS + q * ROWS_PER_CHUNK
            a = max(h0, -dh)                      # first valid output row
            b = min(h0 + ROWS_PER_CHUNK, H - dh)  # one past last valid row
            rhs = x0[:, a + dh:b + dh, co0 + dw:co1 + dw]
            out_ap = psum[:, q, a - h0:b - h0, co0:co1]
            nc.tensor.matmul(out_ap, lhsT, rhs,
                             start=(i == 0), stop=(i == len(taps) - 1))


def _load_x0(nc, x0_pool, g, x_d):
    x0 = x0_pool.tile([128, H, W_SP], BF16, tag="x0")
    nc.gpsimd.dma_start(x0[:, 0:HSPLIT, :],
                        x_d[g * 128:(g + 1) * 128, 0:HSPLIT, :])  # f32->bf16
    nc.gpsimd.dma_start(x0[:, HSPLIT:H, :],
                        x_d[g * 128:(g + 1) * 128, HSPLIT:H, :])
    return x0


def _emit_pe_tile(nc, pools, g, x_d, out_d, wd_sb, x0_pool):
    psum_pool, osb_pool, osbf_pool = pools
    x0 = _load_x0(nc, x0_pool, g, x_d)
    for half in range(2):
        psum = psum_pool.tile([128, HALF_CHUNKS, ROWS_PER_CHUNK, W_SP], F32,
                              tag="psum")
        _emit_pe_taps(nc, psum, g, half, x0, wd_sb, TAPS)
        # f32 eviction + HWDGE store (keeps Sync busy, GpSimd free)
        osb = osbf_pool.tile([128, HALF_CHUNKS * 512], F32, tag="osbf")
        nc.scalar.copy(osb[:], psum[:, :, :, :])
        nc.sync.dma_start(
            out_d[g * 128:(g + 1) * 128,
                  half * HALF_ROWS:(half + 1) * HALF_ROWS, :],
            osb[:],
        )


def _dve_dw0_partial(nc, tmp_pool, part_pool, g, x0, wv_sb):
    """part = sum of the three dw=0 taps (bf16, dense 4x/2x ops only)."""
    cb9 = (g % NBLK) * 9
    part = part_pool.tile([128, H, W_SP], BF16, tag="part")
    nc.vector.tensor_scalar(part[:], x0[:], wv_sb[:, cb9:cb9 + 1], None,
                            mybir.AluOpType.mult)
    for dh in (-1, 1):
        t = TAPS.index((dh, 0))
        wv = wv_sb[:, cb9 + t:cb9 + t + 1]
        tmp = tmp_pool.tile([128, H, W_SP], BF16, tag="tmp")
        nc.vector.tensor_scalar(tmp[:], x0[:], wv, None, mybir.AluOpType.mult)
        oa = max(0, -dh)
        ob = H - max(0, dh)
        nc.vector.tensor_tensor(part[:, oa:ob, :], part[:, oa:ob, :],
                                tmp[:, oa + dh:ob + dh, :],
                                op=mybir.AluOpType.add)
    return part


def _emit_hyb_tile(nc, pools, g, x_d, out_d, wd_sb, wv_sb, x0_pool,
                   tmp_pool, part_pool):
    psum_pool, osb_pool, osbf_pool = pools
    x0 = _load_x0(nc, x0_pool, g, x_d)
    part = _dve_dw0_partial(nc, tmp_pool, part_pool, g, x0, wv_sb)
    for half in range(2):
        psum = psum_pool.tile([128, HALF_CHUNKS, ROWS_PER_CHUNK, W_SP], F32,
                              tag="psum")
        _emit_pe_taps(nc, psum, g, half, x0, wd_sb, DWX_TAPS)
        osb = osb_pool.tile([128, HALF_ROWS, W_SP], BF16, tag="osb")
        nc.scalar.copy(osb[:, :, :], psum[:, :, :, :])
        pv = part[:, half * HALF_ROWS:(half + 1) * HALF_ROWS, :]
        nc.vector.tensor_tensor(osb[:, :, :], osb[:, :, :], pv,
                                op=mybir.AluOpType.add)
        nc.gpsimd.dma_start(
            out_d[g * 128:(g + 1) * 128,
                  half * HALF_ROWS:(half + 1) * HALF_ROWS, :],
            osb[:, :, :],  # bf16 -> f32 cast store
        )


def _emit_dve_tile(nc, pools, g, x_d, out_d, wv_sb, zb_sb, x0_pool):
    """bf16 DVE path for the whole work tile g.

    scalar_tensor_tensor has no fast uop (always 1x), so each tap is
    tensor_scalar (4x mode, dense from offset 0) into a tmp followed by a
    tensor_tensor add (2x) whose read view carries the dh/dw shift (4B
    alignment is enough for 2x).
    """
    x1_pool, tmp_pool, tmp6_pool, odve_pool = pools
    cb = g % NBLK

    x0 = _load_x0(nc, x0_pool, g, x_d)

    # X1[h, 1+w] = x[h, w]; zero cols 0 and 65.  dw=-1 reads cols 0:64
    # (byte offset 0), dw=+1 reads cols 2:66 (byte offset 4).
    x1 = x1_pool.tile([128, H, WPAD], BF16, tag="x1")
    nc.scalar.copy(x1[:, :, 0:1], zb_sb[:])
    nc.scalar.copy(x1[:, :, WPAD - 1:WPAD], zb_sb[:])
    nc.scalar.copy(x1[:, 0:HSPLIT, 1:1 + W_SP], x0[:, 0:HSPLIT, :])
    nc.scalar.copy(x1[:, HSPLIT:H, 1:1 + W_SP], x0[:, HSPLIT:H, :])

    odve = odve_pool.tile([128, H, W_SP], BF16, tag="odve")
    nc.vector.tensor_scalar(odve[:], x0[:], wv_sb[:, cb * 9:cb * 9 + 1], None,
                            mybir.AluOpType.mult)

    for t, (dh, dw) in enumerate(TAPS[1:], start=1):
        wv = wv_sb[:, cb * 9 + t:cb * 9 + t + 1]
        oa = max(0, -dh)          # first valid output row
        ob = H - max(0, dh)       # one past last valid output row
        if dw == 0:
            tmp = tmp_pool.tile([128, H, W_SP], BF16, tag="tmp")
            nc.vector.tensor_scalar(tmp[:], x0[:], wv, None,
                                    mybir.AluOpType.mult)
            in_v = tmp[:, oa + dh:ob + dh, :]
        else:
            tmp6 = tmp6_pool.tile([128, H, WPAD], BF16, tag="tmp6")
            nc.vector.tensor_scalar(tmp6[:], x1[:], wv, None,
                                    mybir.AluOpType.mult)
            col0 = 0 if dw == -1 else 2
            in_v = tmp6[:, oa + dh:ob + dh, col0:col0 + W_SP]
        out_v = odve[:, oa:ob, :]
        nc.vector.tensor_tensor(out_v, out_v, in_v, op=mybir.AluOpType.add)
    nc.gpsimd.dma_start(out_d[g * 128:(g + 1) * 128, :, :], odve[:])  # ->f32


def _build_program(dve_tiles, hyb_tiles):
    nc = bacc.Bacc("TRN2", target_bir_lowering=False, debug=False)
    x_d = nc.dram_tensor("x", [SPC * C, H, W_SP], F32, kind="ExternalInput").ap()
    wd_d = nc.dram_tensor("wd", [128, NBLK * 9 * 128], BF16, kind="ExternalInput").ap()
    wv_d = nc.dram_tensor("wv", [128, NBLK * 9], F32, kind="ExternalInput").ap()
    out_d = nc.dram_tensor("out", [SPC * C, H, W_SP], F32, kind="ExternalOutput").ap()

    with tile.TileContext(nc) as tc:
        with ExitStack() as ctx:
            const_pool = ctx.enter_context(tc.tile_pool(name="const", bufs=1))
            wd_sb = const_pool.tile([128, NBLK * 9 * 128], BF16)
            nc.sync.dma_start(wd_sb[:], wd_d[:])
            wv_sb = const_pool.tile([128, NBLK * 9], F32)
            nc.sync.dma_start(wv_sb[:], wv_d[:])
            zb_sb = const_pool.tile([128, H, 1], BF16)
            nc.vector.memset(zb_sb[:], 0.0)

            psum_pool = ctx.enter_context(tc.tile_pool(name="psum", bufs=2, space="PSUM"))
            osb_pool = ctx.enter_context(tc.tile_pool(name="osb", bufs=3))
            osbf_pool = ctx.enter_context(tc.tile_pool(name="osbf", bufs=3))
            x0_pool = ctx.enter_context(tc.tile_pool(name="x0", bufs=4))
            x1_pool = ctx.enter_context(tc.tile_pool(name="x1", bufs=2))
            tmp_pool = ctx.enter_context(tc.tile_pool(name="tmp", bufs=2))
            tmp6_pool = ctx.enter_context(tc.tile_pool(name="tmp6", bufs=2))
            part_pool = ctx.enter_context(tc.tile_pool(name="part", bufs=2))
            odve_pool = ctx.enter_context(tc.tile_pool(name="odve", bufs=2))
            pe_pools = (psum_pool, osb_pool, osbf_pool)
            dve_pools = (x1_pool, tmp_pool, tmp6_pool, odve_pool)

            for g in range(N_TILES):
                if g in dve_tiles:
                    _emit_dve_tile(nc, dve_pools, g, x_d, out_d, wv_sb, zb_sb,
                                   x0_pool)
                elif g in hyb_tiles:
                    _emit_hyb_tile(nc, pe_pools, g, x_d, out_d, wd_sb, wv_sb,
                                   x0_pool, tmp_pool, part_pool)
                else:
                    _emit_pe_tile(nc, pe_pools, g, x_d, out_d, wd_sb, x0_pool)
    nc.compile()
    return nc


_prog_cache = {}


def _get_program():
    key = (DVE_TILES, HYB_TILES)
    if key not in _prog_cache:
        _prog_cache[key] = _build_program(DVE_TILES, HYB_TILES)
    return _prog_cache[key]


def _host_weights(W):
    wdiag = W[np.arange(C), np.arange(C)]          # [256, 3, 3]
    wd_host = np.zeros((128, NBLK * 9, 128), dtype=np.float32)
    wv_host = np.zeros((128, NBLK * 9), dtype=np.float32)
    r = np.arange(128)
    for cb in range(NBLK):
        for t, (dh, dw) in enumerate(TAPS):
            wd_host[r, cb * 9 + t, r] = wdiag[cb * 128 + r, dh + 1, dw + 1]
            wv_host[r, cb * 9 + t] = wdiag[cb * 128 + r, dh + 1, dw + 1]
    return wd_host.reshape(128, NBLK * 9 * 128).astype(ml_dtypes.bfloat16), wv_host


def _in_maps(x, W):
    wd_host, wv_host = _host_weights(W)
    xs = x.reshape(N_SAMPLES, C, H, W_SP)
    return [
        {
            "x": np.ascontiguousarray(xs[i * SPC:(i + 1) * SPC]).reshape(SPC * C, H, W_SP),
            "wd": wd_host,
            "wv": wv_host,
        }
        for i in range(N_CORES)
    ]


def kernel(x: np.ndarray, W: np.ndarray) -> np.ndarray:
    x = np.ascontiguousarray(x, dtype=np.float32)
    W = np.ascontiguousarray(W, dtype=np.float32)
    assert x.shape == (S, B, C, H, W_SP)
    assert W.shape == (C, C, 3, 3)

    nc = _get_program()
    res = run_bass_kernel_spmd(nc, _in_maps(x, W), core_ids=list(range(N_CORES)))
    out = np.concatenate(
        [res.results[i]["out"].reshape(SPC, C, H, W_SP) for i in range(N_CORES)], axis=0
    )
    return out.reshape(S, B, C, H, W_SP)


# revision 18
# speedup vs baseline: 1.5779x; 1.0151x over previous
"""Depthwise-masked 3x3 conv (eye-masked dense conv) on 8 TRN2 NeuronCores.

Problem: x (2,16,256,64,64) fp32, W (256,256,3,3) fp32; the reference masks W
with eye(C) so only W[c,c,:,:] survives -> depthwise 3x3 "same" conv.

Strategy (per core; data-parallel over the 32 (s,b) samples -> 4 samples/core):
  - channels on partitions: work tile = (sample, channel-block of 128) ->
    bf16 x tile [128, 64, 64] via SWDGE cast-DMA; 8 work tiles per core.
  - PE path: taps as diagonal-stationary bf16 matmuls accumulating in fp32
    PSUM per 512-element bank chunk; boundaries via clipped access patterns
    (bf16 ifmaps allow the odd-width views fp32r rejects).
  - DVE path: per tap, tensor_scalar multiply (4x perf mode, dense from
    offset 0) into a tmp, then tensor_tensor add (2x) with the dh/dw shift
    absorbed into the add's read view; X1 (W-shifted padded copy, built by
    ACT) keeps the column-shifted reads 4B-aligned.
  - hybrid tiles split taps: PE takes the 6 column-shifted taps, DVE takes
    the 3 dw=0 taps and adds its partial onto the evicted PSUM result.
  - rel tolerance is 2e-2; bf16 everywhere lands ~3.5e-3.
"""

import os
from contextlib import ExitStack

import numpy as np
import ml_dtypes

import concourse.bass as bass
import concourse.tile as tile
from concourse import bacc, mybir
from concourse.bass_utils import run_bass_kernel_spmd

S, B, C, H, W_SP = 2, 16, 256, 64, 64
N_CORES = 8
N_SAMPLES = S * B                      # 32
SPC = N_SAMPLES // N_CORES             # 4 samples per core
NBLK = C // 128                        # 2 channel blocks
N_TILES = SPC * NBLK                   # 8 work tiles per core
WPAD = W_SP + 2                        # 66: zero col, 64 data cols, zero col
ROWS_PER_CHUNK = 8                     # 512 fp32 = one PSUM bank
HALF_CHUNKS = 4                        # chunks per half tile (4 banks)
HALF_ROWS = HALF_CHUNKS * ROWS_PER_CHUNK  # 32
HSPLIT = HALF_ROWS + 2                 # x0 half-DMA split row

# center tap first: the start=True matmul covers the full bank
TAPS = [(0, 0), (-1, -1), (-1, 0), (-1, 1), (0, -1), (0, 1), (1, -1), (1, 0), (1, 1)]
DW0_TAPS = [(0, 0), (-1, 0), (1, 0)]                       # DVE side of hybrids
DWX_TAPS = [(-1, -1), (-1, 1), (0, -1), (0, 1), (1, -1), (1, 1)]  # PE side

_DVE_TILES_DEFAULT = "1,5"
_HYB_TILES_DEFAULT = "2,6"
DVE_TILES = frozenset(
    int(v) for v in os.environ.get("KERNEL_DVE_TILES", _DVE_TILES_DEFAULT).split(",")
    if v != ""
)
HYB_TILES = frozenset(
    int(v) for v in os.environ.get("KERNEL_HYB_TILES", _HYB_TILES_DEFAULT).split(",")
    if v != ""
)

F32 = mybir.dt.float32
BF16 = mybir.dt.bfloat16


def _tap_slot(g, t):
    return (g % NBLK) * 9 + t


def _emit_pe_taps(nc, psum, g, half, x0, wd_sb, taps):
    """Diag-matmul the given taps for rows [32*half, ...) into psum.

    Clipped access patterns handle all boundaries: start=True clears the
    whole bank's has_written bits, each element's first writer overwrites,
    later ones accumulate — order independent, so partial-width taps are
    fine as long as every element is covered by some tap.
    """
    for i, (dh, dw) in enumerate(taps):
        t = TAPS.index((dh, dw))
        lhsT = wd_sb[:, _tap_slot(g, t) * 128:(_tap_slot(g, t) + 1) * 128]
        co0 = max(0, -dw)              # first valid output col
        co1 = W_SP - max(0, dw)        # one past last valid output col
        for q in range(HALF_CHUNKS):
            h0 = half * HALF_ROWS + q * ROWS_PER_CHUNK
            a = max(h0, -dh)                      # first valid output row
            b = min(h0 + ROWS_PER_CHUNK, H - dh)  # one past last valid row
            rhs = x0[:, a + dh:b + dh, co0 + dw:co1 + dw]
            out_ap = psum[:, q, a - h0:b - h0, co0:co1]
            nc.tensor.matmul(out_ap, lhsT, rhs,
                             start=(i == 0), stop=(i == len(taps) - 1))


def _load_x0(nc, tc, x0_pool, g, x_d):
    x0 = x0_pool.tile([128, H, W_SP], BF16, tag="x0")
    # stagger issue so early tiles (and output stores) get full DMA bandwidth
    with tc.tile_wait_until(g * 0.007):
        nc.gpsimd.dma_start(x0[:, 0:HSPLIT, :],
                            x_d[g * 128:(g + 1) * 128, 0:HSPLIT, :])  # ->bf16
        nc.gpsimd.dma_start(x0[:, HSPLIT:H, :],
                            x_d[g * 128:(g + 1) * 128, HSPLIT:H, :])
    return x0


def _emit_pe_tile(nc, tc, pools, g, x_d, out_d, wd_sb, x0_pool):
    psum_pool, osb_pool, osbf_pool = pools
    x0 = _load_x0(nc, tc, x0_pool, g, x_d)
    for half in range(2):
        psum = psum_pool.tile([128, HALF_CHUNKS, ROWS_PER_CHUNK, W_SP], F32,
                              tag="psum")
        _emit_pe_taps(nc, psum, g, half, x0, wd_sb, TAPS)
        # f32 eviction + HWDGE store (keeps Sync busy, GpSimd free)
        osb = osbf_pool.tile([128, HALF_CHUNKS * 512], F32, tag="osbf")
        nc.scalar.copy(osb[:], psum[:, :, :, :])
        nc.sync.dma_start(
            out_d[g * 128:(g + 1) * 128,
                  half * HALF_ROWS:(half + 1) * HALF_ROWS, :],
            osb[:],
        )


def _dve_dw0_partial(nc, tmp_pool, part_pool, g, x0, wv_sb):
    """part = sum of the three dw=0 taps (bf16, dense 4x/2x ops only)."""
    cb9 = (g % NBLK) * 9
    part = part_pool.tile([128, H, W_SP], BF16, tag="part")
    nc.vector.tensor_scalar(part[:], x0[:], wv_sb[:, cb9:cb9 + 1], None,
                            mybir.AluOpType.mult)
    for dh in (-1, 1):
        t = TAPS.index((dh, 0))
        wv = wv_sb[:, cb9 + t:cb9 + t + 1]
        tmp = tmp_pool.tile([128, H, W_SP], BF16, tag="tmp")
        nc.vector.tensor_scalar(tmp[:], x0[:], wv, None, mybir.AluOpType.mult)
        oa = max(0, -dh)
        ob = H - max(0, dh)
        nc.vector.tensor_tensor(part[:, oa:ob, :], part[:, oa:ob, :],
                                tmp[:, oa + dh:ob + dh, :],
                                op=mybir.AluOpType.add)
    return part


def _emit_hyb_tile(nc, tc, pools, g, x_d, out_d, wd_sb, wv_sb, x0_pool,
                   tmp_pool, part_pool):
    psum_pool, osb_pool, osbf_pool = pools
    x0 = _load_x0(nc, tc, x0_pool, g, x_d)
    part = _dve_dw0_partial(nc, tmp_pool, part_pool, g, x0, wv_sb)
    for half in range(2):
        psum = psum_pool.tile([128, HALF_CHUNKS, ROWS_PER_CHUNK, W_SP], F32,
                              tag="psum")
        _emit_pe_taps(nc, psum, g, half, x0, wd_sb, DWX_TAPS)
        osb = osb_pool.tile([128, HALF_ROWS, W_SP], BF16, tag="osb")
        nc.scalar.copy(osb[:, :, :], psum[:, :, :, :])
        pv = part[:, half * HALF_ROWS:(half + 1) * HALF_ROWS, :]
        nc.vector.tensor_tensor(osb[:, :, :], osb[:, :, :], pv,
                                op=mybir.AluOpType.add)
        nc.gpsimd.dma_start(
            out_d[g * 128:(g + 1) * 128,
                  half * HALF_ROWS:(half + 1) * HALF_ROWS, :],
            osb[:, :, :],  # bf16 -> f32 cast store
        )


def _emit_dve_tile(nc, tc, pools, g, x_d, out_d, wv_sb, zb_sb, x0_pool):
    """bf16 DVE path for the whole work tile g.

    scalar_tensor_tensor has no fast uop (always 1x), so each tap is
    tensor_scalar (4x mode, dense from offset 0) into a tmp followed by a
    tensor_tensor add (2x) whose read view carries the dh/dw shift (4B
    alignment is enough for 2x).
    """
    x1_pool, tmp_pool, tmp6_pool, odve_pool = pools
    cb = g % NBLK

    x0 = _load_x0(nc, tc, x0_pool, g, x_d)

    # X1[h, 1+w] = x[h, w]; zero cols 0 and 65.  dw=-1 reads cols 0:64
    # (byte offset 0), dw=+1 reads cols 2:66 (byte offset 4).
    x1 = x1_pool.tile([128, H, WPAD], BF16, tag="x1")
    nc.scalar.copy(x1[:, :, 0:1], zb_sb[:])
    nc.scalar.copy(x1[:, :, WPAD - 1:WPAD], zb_sb[:])
    nc.scalar.copy(x1[:, 0:HSPLIT, 1:1 + W_SP], x0[:, 0:HSPLIT, :])
    nc.scalar.copy(x1[:, HSPLIT:H, 1:1 + W_SP], x0[:, HSPLIT:H, :])

    odve = odve_pool.tile([128, H, W_SP], BF16, tag="odve")
    nc.vector.tensor_scalar(odve[:], x0[:], wv_sb[:, cb * 9:cb * 9 + 1], None,
                            mybir.AluOpType.mult)

    for t, (dh, dw) in enumerate(TAPS[1:], start=1):
        wv = wv_sb[:, cb * 9 + t:cb * 9 + t + 1]
        oa = max(0, -dh)          # first valid output row
        ob = H - max(0, dh)       # one past last valid output row
        if dw == 0:
            tmp = tmp_pool.tile([128, H, W_SP], BF16, tag="tmp")
            nc.vector.tensor_scalar(tmp[:], x0[:], wv, None,
                                    mybir.AluOpType.mult)
            in_v = tmp[:, oa + dh:ob + dh, :]
        else:
            tmp6 = tmp6_pool.tile([128, H, WPAD], BF16, tag="tmp6")
            nc.vector.tensor_scalar(tmp6[:], x1[:], wv, None,
                                    mybir.AluOpType.mult)
            col0 = 0 if dw == -1 else 2
            in_v = tmp6[:, oa + dh:ob + dh, col0:col0 + W_SP]
        out_v = odve[:, oa:ob, :]
        nc.vector.tensor_tensor(out_v, out_v, in_v, op=mybir.AluOpType.add)
    nc.gpsimd.dma_start(out_d[g * 128:(g + 1) * 128, :, :], odve[:])  # ->f32


def _build_program(dve_tiles, hyb_tiles):
    nc = bacc.Bacc("TRN2", target_bir_lowering=False, debug=False)
    x_d = nc.dram_tensor("x", [SPC * C, H, W_SP], F32, kind="ExternalInput").ap()
    wd_d = nc.dram_tensor("wd", [128, NBLK * 9 * 128], BF16, kind="ExternalInput").ap()
    wv_d = nc.dram_tensor("wv", [128, NBLK * 9], F32, kind="ExternalInput").ap()
    out_d = nc.dram_tensor("out", [SPC * C, H, W_SP], F32, kind="ExternalOutput").ap()

    with tile.TileContext(nc) as tc:
        with ExitStack() as ctx:
            const_pool = ctx.enter_context(tc.tile_pool(name="const", bufs=1))
            wd_sb = const_pool.tile([128, NBLK * 9 * 128], BF16)
            nc.sync.dma_start(wd_sb[:], wd_d[:])
            wv_sb = const_pool.tile([128, NBLK * 9], F32)
            nc.sync.dma_start(wv_sb[:], wv_d[:])
            zb_sb = const_pool.tile([128, H, 1], BF16)
            nc.vector.memset(zb_sb[:], 0.0)

            psum_pool = ctx.enter_context(tc.tile_pool(name="psum", bufs=2, space="PSUM"))
            osb_pool = ctx.enter_context(tc.tile_pool(name="osb", bufs=3))
            osbf_pool = ctx.enter_context(tc.tile_pool(name="osbf", bufs=4))
            x0_pool = ctx.enter_context(tc.tile_pool(name="x0", bufs=5))
            x1_pool = ctx.enter_context(tc.tile_pool(name="x1", bufs=2))
            tmp_pool = ctx.enter_context(tc.tile_pool(name="tmp", bufs=2))
            tmp6_pool = ctx.enter_context(tc.tile_pool(name="tmp6", bufs=2))
            part_pool = ctx.enter_context(tc.tile_pool(name="part", bufs=2))
            odve_pool = ctx.enter_context(tc.tile_pool(name="odve", bufs=2))
            pe_pools = (psum_pool, osb_pool, osbf_pool)
            dve_pools = (x1_pool, tmp_pool, tmp6_pool, odve_pool)

            for g in range(N_TILES):
                if g in dve_tiles:
                    _emit_dve_tile(nc, tc, dve_pools, g, x_d, out_d, wv_sb,
                                   zb_sb, x0_pool)
                elif g in hyb_tiles:
                    _emit_hyb_tile(nc, tc, pe_pools, g, x_d, out_d, wd_sb,
                                   wv_sb, x0_pool, tmp_pool, part_pool)
                else:
                    _emit_pe_tile(nc, tc, pe_pools, g, x_d, out_d, wd_sb, x0_pool)
    nc.compile()
    return nc


_prog_cache = {}


def _get_program():
    key = (DVE_TILES, HYB_TILES)
    if key not in _prog_cache:
        _prog_cache[key] = _build_program(DVE_TILES, HYB_TILES)
    return _prog_cache[key]


def _host_weights(W):
    wdiag = W[np.arange(C), np.arange(C)]          # [256, 3, 3]
    wd_host = np.zeros((128, NBLK * 9, 128), dtype=np.float32)
    wv_host = np.zeros((128, NBLK * 9), dtype=np.float32)
    r = np.arange(128)
    for cb in range(NBLK):
        for t, (dh, dw) in enumerate(TAPS):
            wd_host[r, cb * 9 + t, r] = wdiag[cb * 128 + r, dh + 1, dw + 1]
            wv_host[r, cb * 9 + t] = wdiag[cb * 128 + r, dh + 1, dw + 1]
    return wd_host.reshape(128, NBLK * 9 * 128).astype(ml_dtypes.bfloat16), wv_host


def _in_maps(x, W):
    wd_host, wv_host = _host_weights(W)
    xs = x.reshape(N_SAMPLES, C, H, W_SP)
    return [
        {
            "x": np.ascontiguousarray(xs[i * SPC:(i + 1) * SPC]).reshape(SPC * C, H, W_SP),
            "wd": wd_host,
            "wv": wv_host,
        }
        for i in range(N_CORES)
    ]


def kernel(x: np.ndarray, W: np.ndarray) -> np.ndarray:
    x = np.ascontiguousarray(x, dtype=np.float32)
    W = np.ascontiguousarray(W, dtype=np.float32)
    assert x.shape == (S, B, C, H, W_SP)
    assert W.shape == (C, C, 3, 3)

    nc = _get_program()
    res = run_bass_kernel_spmd(nc, _in_maps(x, W), core_ids=list(range(N_CORES)))
    out = np.concatenate(
        [res.results[i]["out"].reshape(SPC, C, H, W_SP) for i in range(N_CORES)], axis=0
    )
    return out.reshape(S, B, C, H, W_SP)
